# revision 5
# baseline (speedup 1.0000x reference)
"""Channel-attention MultiHeadAttention kernel for Trainium2 (8 NeuronCores).

Math: for this module, attention is over channels (d x d per head) with the
spatial dim N = H*W as the contraction axis. The whole module collapses:
  G = x @ x.T (256x256 Gram), s = rowsum(x)
  S = scale * [Wq|bq] @ [[G, s],[s^T, N]] @ [Wk|bk]^T   (only 8 diag 32x32 blocks)
  attn = softmax(S_blocks)
  Wfinal = WoutP @ blockdiag(attn) @ Wv ;  bfinal = bout + WoutP @ blockdiag(attn) @ bv
  out = Wfinal @ x + bfinal
Sharding: data-parallel over batch B=8, one batch element per core. No collectives.
"""

import numpy as np
from contextlib import ExitStack

B, C, H, W = 8, 256, 128, 128
N = H * W          # 16384
NH, D = 8, 32      # heads, head dim
SCALE = D ** -0.5
CH = 512           # phase A/C column chunk
SUB = 128          # transpose subchunk
NCORES = 8

TRACE = False      # test.py may set kernel.TRACE = True
LAST_RESULTS = {}  # exec_time_ns etc. for test.py

_CACHE = {}


def _build_real():
    import concourse.bacc as bacc
    import concourse.mybir as mybir
    import concourse.tile as tile

    dt = mybir.dt
    f32, f16 = dt.float32, dt.float16
    Exp = mybir.ActivationFunctionType.Exp
    X = mybir.AxisListType.X

    nc = bacc.Bacc(trn_type="TRN2")

    x_d = nc.dram_tensor("xb", [C, N], f32, kind="ExternalInput")
    qaT_d = nc.dram_tensor("qaT", [257, 256], f32, kind="ExternalInput")
    kaT_d = nc.dram_tensor("kaT", [257, 256], f32, kind="ExternalInput")
    wva_d = nc.dram_tensor("wva", [256, 257], f32, kind="ExternalInput")
    wpT_d = nc.dram_tensor("wpT", [256, 256], f32, kind="ExternalInput")
    bout_d = nc.dram_tensor("boutc", [256, 1], f32, kind="ExternalInput")
    id_d = nc.dram_tensor("ident", [128, 128], f32, kind="ExternalInput")
    out_d = nc.dram_tensor("out", [C, N], f32, kind="ExternalOutput")

    with ExitStack() as top:
        tc = top.enter_context(tile.TileContext(nc))
        persist = top.enter_context(tc.tile_pool(name="persist", bufs=1))

        x16 = [persist.tile([128, N], f16, tag=f"x16_{i}", name=f"x16_{i}") for i in range(2)]

        qaT_t = [persist.tile([128, 256], f32, tag="qaT0", name="qaT0"),
                 persist.tile([128, 256], f32, tag="qaT1", name="qaT1"),
                 persist.tile([1, 256], f32, tag="qaT2", name="qaT2")]
        kaT_t = [persist.tile([128, 256], f32, tag="kaT0", name="kaT0"),
                 persist.tile([128, 256], f32, tag="kaT1", name="kaT1"),
                 persist.tile([1, 256], f32, tag="kaT2", name="kaT2")]
        wva_t = [persist.tile([128, 257], f32, tag="wva0", name="wva0"),
                 persist.tile([128, 257], f32, tag="wva1", name="wva1")]
        wpT_t = [persist.tile([128, 256], f32, tag="wpT0", name="wpT0"),
                 persist.tile([128, 256], f32, tag="wpT1", name="wpT1")]
        bout_t = [persist.tile([128, 1], f32, tag="bout0", name="bout0"),
                  persist.tile([128, 1], f32, tag="bout1", name="bout1")]
        id_t = persist.tile([128, 128], f32, tag="ident", name="ident")

        nc.sync.dma_start(out=qaT_t[0], in_=qaT_d.ap()[0:128, :])
        nc.sync.dma_start(out=qaT_t[1], in_=qaT_d.ap()[128:256, :])
        nc.sync.dma_start(out=qaT_t[2], in_=qaT_d.ap()[256:257, :])
        nc.sync.dma_start(out=kaT_t[0], in_=kaT_d.ap()[0:128, :])
        nc.sync.dma_start(out=kaT_t[1], in_=kaT_d.ap()[128:256, :])
        nc.sync.dma_start(out=kaT_t[2], in_=kaT_d.ap()[256:257, :])
        nc.sync.dma_start(out=wva_t[0], in_=wva_d.ap()[0:128, :])
        nc.sync.dma_start(out=wva_t[1], in_=wva_d.ap()[128:256, :])
        nc.sync.dma_start(out=wpT_t[0], in_=wpT_d.ap()[0:128, :])
        nc.sync.dma_start(out=wpT_t[1], in_=wpT_d.ap()[128:256, :])
        nc.sync.dma_start(out=bout_t[0], in_=bout_d.ap()[0:128, :])
        nc.sync.dma_start(out=bout_t[1], in_=bout_d.ap()[128:256, :])
        nc.sync.dma_start(out=id_t, in_=id_d.ap())

        # outputs of the tiny stage used by phase C
        wf16 = [persist.tile([128, 256], f16, tag=f"wf16_{k}", name=f"wf16_{k}") for k in range(2)]
        bf_col = [persist.tile([128, 1], f32, tag=f"bf{m}", name=f"bf{m}") for m in range(2)]

        # ---------------- Phase A: Gram accumulation ----------------
        with ExitStack() as sA:
            stage = sA.enter_context(tc.tile_pool(name="stage", bufs=3))
            psA = sA.enter_context(tc.tile_pool(name="psA", bufs=1, space="PSUM"))

            G_ps = [psA.tile([128, 257], f32, tag=f"g{i}", name=f"g{i}") for i in range(2)]

            # explicit ring of transpose-target tiles (ones column pre-set)
            NXT = 4
            xts = [persist.tile([128, 257], f16, tag=f"xt{j}", name=f"xt{j}") for j in range(NXT)]
            for j in range(NXT):
                nc.vector.memset(xts[j][:, 256:257], 1.0)

            t = 0
            for co in range(N // CH):
                xs = [stage.tile([128, CH], f32, tag=f"xs{i}", name=f"xs{i}") for i in range(2)]
                sl = slice(co * CH, (co + 1) * CH)
                nc.sync.dma_start(out=xs[0], in_=x_d.ap()[0:128, sl])
                nc.sync.dma_start(out=xs[1], in_=x_d.ap()[128:256, sl])
                for i in range(2):
                    nc.vector.tensor_copy(out=x16[i][:, sl], in_=xs[i][:])
                for ci in range(CH // SUB):
                    n0 = co * CH + ci * SUB
                    xt = xts[t % NXT]
                    t += 1
                    nc.scalar.dma_start(out=xt[:, 0:128],
                                        in_=x16[0][:, n0:n0 + SUB], transpose=True)
                    nc.scalar.dma_start(out=xt[:, 128:256],
                                        in_=x16[1][:, n0:n0 + SUB], transpose=True)
                    first, last = (n0 == 0), (n0 == N - SUB)
                    nc.tensor.matmul(G_ps[0][:], lhsT=xt[:, 0:128], rhs=xt[:],
                                     start=first, stop=last)
                    nc.tensor.matmul(G_ps[1][:], lhsT=xt[:, 128:256], rhs=xt[:],
                                     start=first, stop=last)

            # ---------------- Tiny stage (all fp32, exact) ----------------
            tp = sA.enter_context(tc.tile_pool(name="tinysb", bufs=1))
            pst = sA.enter_context(tc.tile_pool(name="tinyps", bufs=2, space="PSUM"))

            # Ga tiles: [G | s] rows 0:128, 128:256 plus the [s^T | N] row
            Ga = [tp.tile([128, 257], f32, tag=f"Ga{k}", name=f"Ga{k}") for k in range(2)]
            for k in range(2):
                nc.vector.tensor_copy(out=Ga[k][:], in_=G_ps[k][:])
            Ga2 = tp.tile([1, 257], f32, tag="Ga2", name="Ga2")
            for k in range(2):
                srow_ps = pst.tile([1, 128], f32, tag="tinyps", name="tinyps")
                nc.tensor.transpose(srow_ps[:], Ga[k][:, 256:257], id_t[:])
                nc.vector.tensor_copy(out=Ga2[0:1, 128 * k:128 * (k + 1)],
                                      in_=srow_ps[:])
            nc.vector.memset(Ga2[0:1, 256:257], float(N))
            GaK = [Ga[0], Ga[1], Ga2]

            # T2 = Ga @ KaT  (257 x 256), M-tiles over rows of T2
            t2s = [tp.tile([128, 256], f32, tag="t2s0", name="t2s0"),
                   tp.tile([128, 256], f32, tag="t2s1", name="t2s1"),
                   tp.tile([1, 256], f32, tag="t2s2", name="t2s2")]
            for m in range(3):
                msl = slice(256, 257) if m == 2 else slice(128 * m, 128 * (m + 1))
                t2_ps = pst.tile([1 if m == 2 else 128, 256], f32, tag="tinyps", name="tinyps")
                for k in range(3):
                    nc.tensor.matmul(t2_ps[:], lhsT=GaK[k][:, msl], rhs=kaT_t[k][:],
                                     start=(k == 0), stop=(k == 2))
                nc.vector.tensor_copy(out=t2s[m][:], in_=t2_ps[:])

            # S_full = QaT.T @ T2 (256 x 256) in PSUM
            SF = []
            for m in range(2):
                sf_ps = pst.tile([128, 256], f32, tag="tinyps", name=f"sfps{m}")
                msl = slice(128 * m, 128 * (m + 1))
                for k in range(3):
                    nc.tensor.matmul(sf_ps[:], lhsT=qaT_t[k][:, msl], rhs=t2s[k][:],
                                     start=(k == 0), stop=(k == 2))
                SF.append(sf_ps)

            # extract 8 diagonal 32x32 blocks -> S_stack (2 x (128, 32))
            Sst = [tp.tile([128, 32], f32, tag=f"sst{q}", name=f"sst{q}") for q in range(2)]
            for h in range(NH):
                q, po = h // 4, (h % 4) * 32
                nc.vector.tensor_copy(
                    out=Sst[q][po:po + 32, :],
                    in_=SF[q][po:po + 32, h * 32:(h + 1) * 32])

            # softmax over free dim (32)
            att = [tp.tile([128, 32], f32, tag=f"att{q}", name=f"att{q}") for q in range(2)]
            for q in range(2):
                nm = tp.tile([128, 1], f32, tag=f"nm{q}", name=f"nm{q}")
                nc.vector.reduce_max(out=nm[:], in_=Sst[q][:], axis=X, negate=True)
                ex = tp.tile([128, 32], f32, tag=f"ex{q}", name=f"ex{q}")
                nc.scalar.activation(out=ex[:], in_=Sst[q][:], func=Exp,
                                     bias=nm[:], scale=1.0)
                sm = tp.tile([128, 1], f32, tag=f"sm{q}", name=f"sm{q}")
                nc.vector.reduce_sum(out=sm[:], in_=ex[:], axis=X)
                rc = tp.tile([128, 1], f32, tag=f"rc{q}", name=f"rc{q}")
                nc.vector.reciprocal(out=rc[:], in_=sm[:])
                nc.vector.tensor_scalar_mul(att[q][:], ex[:], rc[:])

            # block-diagonal attn, then transpose the two diagonal quadrants
            abd = [tp.tile([128, 128], f32, tag=f"abd{q}", name=f"abd{q}") for q in range(2)]
            for q in range(2):
                nc.vector.memset(abd[q][:], 0.0)
            for h in range(NH):
                q, po = h // 4, (h % 4) * 32
                nc.vector.tensor_copy(out=abd[q][po:po + 32, po:po + 32],
                                      in_=att[q][po:po + 32, :])
            abdT = [tp.tile([128, 128], f32, tag=f"abdT{q}", name=f"abdT{q}") for q in range(2)]
            for q in range(2):
                tq_ps = pst.tile([128, 128], f32, tag="tinyps", name="tinyps")
                nc.tensor.transpose(tq_ps[:], abd[q][:], id_t[:])
                nc.vector.tensor_copy(out=abdT[q][:], in_=tq_ps[:])

            # Weff_aug = blockdiag(attn) @ [Wv | bv]  (256 x 257)
            weff = [tp.tile([128, 257], f32, tag=f"weff{k}", name=f"weff{k}") for k in range(2)]
            for k in range(2):
                we_ps = pst.tile([128, 257], f32, tag="tinyps", name="tinyps")
                nc.tensor.matmul(we_ps[:], lhsT=abdT[k][:], rhs=wva_t[k][:],
                                 start=True, stop=True)
                nc.vector.tensor_copy(out=weff[k][:], in_=we_ps[:])

            # Wfinal^T = Weff[:, :256] as lhsT against WoutP^T; cast to fp16
            for m in range(2):
                msl = slice(128 * m, 128 * (m + 1))
                wf_ps = pst.tile([128, 256], f32, tag="tinyps", name=f"wfps{m}")
                for k in range(2):
                    nc.tensor.matmul(wf_ps[:], lhsT=weff[k][:, msl], rhs=wpT_t[k][:],
                                     start=(k == 0), stop=(k == 1))
                nc.vector.tensor_copy(out=wf16[m][:], in_=wf_ps[:])

            # bfinal = bout + WoutP @ beff   (beff = Weff[:, 256])
            for m in range(2):
                msl = slice(128 * m, 128 * (m + 1))
                bf_ps = pst.tile([128, 1], f32, tag="tinyps", name="tinyps")
                for k in range(2):
                    nc.tensor.matmul(bf_ps[:], lhsT=wpT_t[k][:, msl],
                                     rhs=weff[k][:, 256:257],
                                     start=(k == 0), stop=(k == 1))
                nc.vector.tensor_add(bf_col[m][:], bf_ps[:], bout_t[m][:])

        # ---------------- Phase C: out = Wfinal @ x + bfinal ----------------
        with ExitStack() as sC:
            ost = sC.enter_context(tc.tile_pool(name="ost", bufs=3))
            psC = sC.enter_context(tc.tile_pool(name="psC", bufs=4, space="PSUM"))
            for co in range(N // CH):
                sl = slice(co * CH, (co + 1) * CH)
                for m in range(2):
                    o_ps = psC.tile([128, CH], f32, tag="ops", name="ops")
                    for k in range(2):
                        nc.tensor.matmul(o_ps[:], lhsT=wf16[k][:, 128 * m:128 * (m + 1)],
                                         rhs=x16[k][:, sl],
                                         start=(k == 0), stop=(k == 1))
                    o_sb = ost.tile([128, CH], f32, tag=f"osb{m}", name=f"osb{m}")
                    nc.vector.tensor_scalar_add(o_sb[:], o_ps[:], bf_col[m][:])
                    nc.sync.dma_start(out=out_d.ap()[128 * m:128 * (m + 1), sl],
                                      in_=o_sb[:])

    nc.finalize()
    return nc


def _host_prep(Wqkv, bqkv, Wout, bout):
    Wq, Wk, Wv = Wqkv[:C], Wqkv[C:2 * C], Wqkv[2 * C:]
    bq, bk, bv = bqkv[:C], bqkv[C:2 * C], bqkv[2 * C:]
    qaT = np.concatenate([Wq.T, bq[None, :]], axis=0) * SCALE   # (257, 256)
    kaT = np.concatenate([Wk.T, bk[None, :]], axis=0)           # (257, 256)
    wva = np.concatenate([Wv, bv[:, None]], axis=1)             # (256, 257)
    r = np.arange(C)
    WoutP = Wout[:, (r % D) * NH + (r // D)]                    # (256, 256)
    wpT = np.ascontiguousarray(WoutP.T)
    return (np.ascontiguousarray(qaT, dtype=np.float32),
            np.ascontiguousarray(kaT, dtype=np.float32),
            np.ascontiguousarray(wva, dtype=np.float32),
            np.ascontiguousarray(wpT, dtype=np.float32),
            np.ascontiguousarray(bout[:, None], dtype=np.float32),
            np.eye(128, dtype=np.float32))


def kernel(x, Wqkv, bqkv, Wout, bout, num_heads):
    from concourse.bass_utils import run_bass_kernel_spmd

    assert int(num_heads) == NH
    x = np.ascontiguousarray(np.asarray(x), dtype=np.float32)
    qaT, kaT, wva, wpT, boutc, ident = _host_prep(
        np.asarray(Wqkv, dtype=np.float32), np.asarray(bqkv, dtype=np.float32),
        np.asarray(Wout, dtype=np.float32), np.asarray(bout, dtype=np.float32))

    if "nc" not in _CACHE:
        _CACHE["nc"] = _build_real()
    nc = _CACHE["nc"]

    shared = {"qaT": qaT, "kaT": kaT, "wva": wva, "wpT": wpT,
              "boutc": boutc, "ident": ident}
    in_maps = [{"xb": np.ascontiguousarray(x[c].reshape(C, N)), **shared}
               for c in range(NCORES)]

    res = run_bass_kernel_spmd(nc, in_maps, core_ids=list(range(NCORES)),
                               trace=TRACE)
    LAST_RESULTS["exec_time_ns"] = res.exec_time_ns
    out = np.stack([res.results[c]["out"] for c in range(NCORES)])
    return out.reshape(B, C, H, W)
